# revision 30
# baseline (speedup 1.0000x reference)
"""Trainium2 Bass kernel for the per-cell-MLP "MAR one-sided missingness" model.

Model (per cell (n,t) of a 1024x128 grid):
    xc     = X[n, col_idx[n,t]]
    h      = relu(W_in[n,t,:,0]*xc + W_in[n,t,:,1]*X[n,t] + b_in[n,t,:])   # [H]
    out    = sigmoid(dot(W_out[n,t,:], h) + b_out[n,t])

Sharding: rows N split across 8 cores (128 rows each), fully data parallel.

Memory-bound: the four weight tensors stream as f16 (rel err 1.1e-2 on HW
vs the 2e-2 gate), halving HBM traffic vs f32. The neighbor gather is
factored as col_idx = 16*q + r: a [16 x T] f8 mask per row gathers 8
candidate values per cell on the PE (K=16 contraction), and an f16
one-hot-over-q selector (packed into the weight stream) picks among them
with a tiny multiply+reduce - ~0.55MB of masks instead of 2.1MB of full
one-hots. Per 16-row block the weights+selectors ride ONE ~2MB DMA (the two
middle pairs of blocks share 4MB DMAs); blocks alternate between the two
HWDGE queues (sync/scalar) so per-DMA init latency overlaps the other
queue's transfer, and deep tile pools let DMA run well ahead of compute.
Block 0 is split in two so compute starts after half the bytes.

Engine placement is constrained by the TRN2 ISA: the Pool/GPSIMD engine
implements only plain tensor_tensor add/mult (no TensorScalarPtr opcode, no
max ALU op), and runs ~2x slower per element than DVE, so it carries exactly
one add per block; relu lives fused in the DVE per-row STT, and the
u = a0 + v add runs on the otherwise-idle PE as a pair of identity matmuls
accumulating into PSUM (also giving an exact f32 pre-activation).

Per-core layout: partition dim = t (128 cells of one row), free dim = (g,h).
Per block s (rows n0..n0+15):
  PE   : Y[:, 8g:8g+8] = R_n^T @ xq_n     (stage-1 gather, f8 x f16, K=16)
  DVE  : xc = sum_k Q * Y                  (stage-2 select, from PSUM)
  ACT  : per row: a0_g = w0_g * xc_g       (per-partition scale)
  DVE  : m1 = w1 * x_bc                   (broadcast over h)
  GPS  : v  = m1 + b
  PE   : u  = I@a0 + I@v accumulated in PSUM (8-row chunks)
  DVE  : per-row STT relu(u_g)*wo_g from PSUM, accum_out -> red[:, n]
         (the accum_out free-dim sum IS the h-reduction -> logit)
Epilogue: out = sigmoid(red + b_out^T), DMA out, host transposes back.
"""

import ml_dtypes
import numpy as np

N, T, H = 1024, 128, 128
M = 8            # cores
NR = N // M      # rows per core
G = 16           # rows per block
S = NR // G      # blocks
GH = G * H
QSEG = G * 8              # f16 one-hot-over-q selectors per block
BSEG = 4 * GH + QSEG      # f16 elems of w1|b|w0|wo|Q per block
# block segment layout (f16 offsets)
OFF_W1 = 0
OFF_B = GH
OFF_W0 = 2 * GH
OFF_WO = 3 * GH
OFF_Q = 4 * GH
SPLIT0 = OFF_W0           # block 0 first-DMA end (enables m1/v)

_cache = {}


def _build():
    if "nc" in _cache:
        return _cache["nc"]
    import concourse.bacc as bacc
    import concourse.mybir as mybir
    import concourse.tile as tile

    f32 = mybir.dt.float32
    f16 = mybir.dt.float16
    f8 = mybir.dt.float8e4
    Alu = mybir.AluOpType
    Act = mybir.ActivationFunctionType

    nc = bacc.Bacc()
    wall = nc.declare_dram_parameter("wall", [T, S * BSEG], f16, isOutput=False)
    xf16 = nc.declare_dram_parameter("xf16", [T, NR], f16, isOutput=False)
    ident = nc.declare_dram_parameter("ident", [T, T], f8, isOutput=False)
    rall = nc.declare_dram_parameter("rall", [16, NR * T], f8, isOutput=False)
    xq16 = nc.declare_dram_parameter("xq16", [16, 8 * NR], f16, isOutput=False)
    bout = nc.declare_dram_parameter("bout", [T, NR], f32, isOutput=False)
    out = nc.declare_dram_parameter("out", [T, NR], f32, isOutput=True)

    with tile.TileContext(nc) as tc:
        with (
            tc.tile_pool(name="const", bufs=1) as constp,
            tc.tile_pool(name="w0p", bufs=1) as w0p,
            tc.tile_pool(name="wp", bufs=1) as wp,
            tc.tile_pool(name="wpp", bufs=1) as wpp,
            tc.tile_pool(name="front", bufs=2) as frontp,
            tc.tile_pool(name="backp", bufs=2) as backp,
            tc.tile_pool(name="rrow", bufs=1) as rrowp,
            tc.tile_pool(name="acc", bufs=1) as accp,
            tc.tile_pool(name="xcp", bufs=2) as xcp,
            tc.tile_pool(name="psxc", bufs=2, space="PSUM") as psxcp,
            tc.tile_pool(name="psu", bufs=2, space="PSUM") as psup,
        ):
            xf_sb = constp.tile([T, NR], f16)
            nc.scalar.dma_start(xf_sb[:], xf16[:])
            id_sb = constp.tile([T, T], f8)
            nc.scalar.dma_start(id_sb[:], ident[:])
            r_sb = constp.tile([16, NR * T], f8)
            nc.scalar.dma_start(r_sb[:], rall[:])
            xq_sb = constp.tile([16, 8 * NR], f16)
            nc.scalar.dma_start(xq_sb[:], xq16[:])
            bo_sb = constp.tile([T, NR], f32)

            red = accp.tile([T, NR], f32)
            rsc = rrowp.tile([T, H], f16)

            state = {}

            # every block resident in its own buffer: emit ALL weight DMAs
            # up front so no dma_start ever blocks an engine sequencer
            # behind data-dependent compute.
            blk = {}
            wA = w0p.tile([T, SPLIT0], f16, tag="wA")
            nc.sync.dma_start(wA[:], wall[:, 0:SPLIT0])
            wB = w0p.tile([T, BSEG - SPLIT0], f16, tag="wB")
            nc.sync.dma_start(wB[:], wall[:, SPLIT0:BSEG])
            blk[0] = (wA, wB)
            for s, qeng in ((1, nc.scalar), (6, nc.sync), (7, nc.scalar)):
                wt = wp.tile([T, BSEG], f16, tag=f"w{s}")
                qeng.dma_start(wt[:], wall[:, s * BSEG : (s + 1) * BSEG])
                blk[s] = wt
            for s, qeng in ((2, nc.sync), (4, nc.scalar)):
                wpr = wpp.tile([T, 2 * BSEG], f16, tag=f"wp{s}")
                qeng.dma_start(
                    wpr[:], wall[:, s * BSEG : (s + 2) * BSEG]
                )
                blk[s] = wpr[:, 0:BSEG]
                blk[s + 1] = wpr[:, BSEG : 2 * BSEG]

            def front(s):
                n0 = s * G
                nsl = slice(n0, n0 + G)
                if s == 0:
                    wA, wB = blk[0]
                    w1s = wA[:, OFF_W1 : OFF_W1 + GH]
                    bs = wA[:, OFF_B : OFF_B + GH]
                    w0s = wB[:, 0:GH]
                    wos = wB[:, GH : 2 * GH]
                    qs = wB[:, 2 * GH : 2 * GH + QSEG]
                else:
                    w = blk[s]
                    w1s = w[:, OFF_W1 : OFF_W1 + GH]
                    bs = w[:, OFF_B : OFF_B + GH]
                    w0s = w[:, OFF_W0 : OFF_W0 + GH]
                    wos = w[:, OFF_WO : OFF_WO + GH]
                    qs = w[:, OFF_Q : OFF_Q + QSEG]

                y_ps = psxcp.tile([T, G * 8], f32, tag="y")
                for g in range(G):
                    n = n0 + g
                    nc.tensor.matmul(
                        y_ps[:, g * 8 : (g + 1) * 8],
                        r_sb[:, n * T : (n + 1) * T],
                        xq_sb[:, 8 * n : 8 * n + 8],
                        start=True,
                        stop=True,
                    )
                qy = xcp.tile([T, G * 8], f32, tag="qy")
                nc.vector.tensor_tensor(qy[:], y_ps[:], qs, Alu.mult)
                xc_sb = xcp.tile([T, G], f32, tag="xcsb")
                nc.vector.tensor_reduce(
                    xc_sb[:],
                    qy[:].rearrange("p (g k) -> p g k", g=G),
                    axis=mybir.AxisListType.X,
                    op=Alu.add,
                )

                m1 = frontp.tile([T, GH], f16, tag="m1")
                nc.vector.tensor_tensor(
                    m1[:].rearrange("p (g h) -> p g h", g=G),
                    w1s.rearrange("p (g h) -> p g h", g=G),
                    xf_sb[:, nsl].broadcast_to([T, G, H]),
                    Alu.mult,
                )
                v = frontp.tile([T, GH], f16, tag="v")
                nc.gpsimd.tensor_tensor(v[:], m1[:], bs, Alu.add)
                a0 = frontp.tile([T, GH], f16, tag="a0")
                for g in range(G):
                    nc.scalar.activation(
                        a0[:, g * H : (g + 1) * H],
                        w0s[:, g * H : (g + 1) * H],
                        Act.Copy,
                        scale=xc_sb[:, g : g + 1],
                    )
                state[s] = (n0, v, a0, wos)

            def back(s):
                n0, v, a0, wos = state.pop(s)
                # u = a0 + v accumulated on the (otherwise idle) PE via
                # identity matmuls into PSUM, in 8-row chunks; the per-row
                # STT reads u straight from PSUM.
                for h0 in range(0, G, G // 2):
                    ups = psup.tile([T, GH // 2], f32, tag="ups")
                    for c0 in range(0, GH // 2, 512):
                        as_ = slice(h0 * H + c0, h0 * H + c0 + 512)
                        nc.tensor.matmul(
                            ups[:, c0 : c0 + 512], id_sb[:], a0[:, as_],
                            start=True, stop=False,
                        )
                        nc.tensor.matmul(
                            ups[:, c0 : c0 + 512], id_sb[:], v[:, as_],
                            start=False, stop=True,
                        )
                    for g in range(h0, h0 + G // 2):
                        n = n0 + g
                        nc.vector.scalar_tensor_tensor(
                            rsc[:],
                            ups[:, (g - h0) * H : (g - h0 + 1) * H],
                            0.0,
                            wos[:, g * H : (g + 1) * H],
                            Alu.max,
                            Alu.mult,
                            accum_out=red[:, n : n + 1],
                        )

            for stage in range(S + 1):
                if stage < S:
                    front(stage)
                if stage >= 1:
                    back(stage - 1)

            nc.scalar.dma_start(bo_sb[:], bout[:])
            lg = backp.tile([T, NR], f32, tag="lg")
            nc.vector.tensor_tensor(lg[:], red[:], bo_sb[:], Alu.add)
            ot = backp.tile([T, NR], f32, tag="ot")
            nc.scalar.activation(ot[:], lg[:], Act.Sigmoid)
            nc.sync.dma_start(out[:], ot[:])

    nc.compile()
    _cache["nc"] = nc
    return nc


def make_in_maps(X, W_in, b_in, W_out, b_out, col_idx):
    X = np.asarray(X, dtype=np.float32)
    W_in = np.asarray(W_in, dtype=np.float32)
    b_in = np.asarray(b_in, dtype=np.float32)
    W_out = np.asarray(W_out, dtype=np.float32)
    b_out = np.asarray(b_out, dtype=np.float32)
    col_idx = np.asarray(col_idx)

    jj = np.arange(T)
    in_maps = []
    for c in range(M):
        sl = slice(c * NR, (c + 1) * NR)
        Wc = W_in[sl]  # [NR, T, H, 2]

        wall = np.empty((T, S, BSEG), np.float16)

        def seg(off):
            return wall[:, :, off : off + GH].reshape(T, S, G, H)

        seg(OFF_W1)[:] = Wc[:, :, :, 1].transpose(1, 0, 2).reshape(T, S, G, H)
        seg(OFF_B)[:] = b_in[sl].transpose(1, 0, 2).reshape(T, S, G, H)
        seg(OFF_W0)[:] = Wc[:, :, :, 0].transpose(1, 0, 2).reshape(T, S, G, H)
        seg(OFF_WO)[:] = W_out[sl].transpose(1, 0, 2).reshape(T, S, G, H)

        # two-stage gather masks: c = 16*q + r.  R[j, n*T+t] = (r(n,t)==j)
        # selects 8 candidates per cell on PE; Q[t, (g,k)] = (q(n,t)==k)
        # (f16, packed into the block segment) picks among them.
        cc = col_idx[sl]
        rmask = ((cc % 16).reshape(1, -1) == np.arange(16)[:, None]).astype(
            ml_dtypes.float8_e4m3
        )  # [16, NR*T]
        qsel = (
            (cc // 16).T.reshape(T, S, G, 1) == np.arange(8).reshape(1, 1, 1, 8)
        ).astype(np.float16)  # [T, S, G, 8]
        wall[:, :, OFF_Q : OFF_Q + QSEG] = qsel.reshape(T, S, QSEG)

        xtc = np.ascontiguousarray(X[sl].T)  # [t, n] f32
        idm = np.eye(T, dtype=np.float32).astype(ml_dtypes.float8_e4m3)
        xq = np.ascontiguousarray(
            X[sl].reshape(NR, 8, 16).transpose(2, 0, 1).reshape(16, NR * 8)
        ).astype(np.float16)

        in_maps.append(
            {
                "wall": np.ascontiguousarray(wall.reshape(T, S * BSEG)),
                "xf16": xtc.astype(np.float16),
                "ident": idm,
                "rall": rmask,
                "xq16": xq,
                "bout": np.ascontiguousarray(b_out[sl].T),
            }
        )
    return in_maps


def kernel(X, W_in, b_in, W_out, b_out, col_idx):
    from concourse.bass_utils import run_bass_kernel_spmd

    nc = _build()
    in_maps = make_in_maps(X, W_in, b_in, W_out, b_out, col_idx)
    res = run_bass_kernel_spmd(nc, in_maps, list(range(M))).results
    out = np.empty((N, T), np.float32)
    for c in range(M):
        out[c * NR : (c + 1) * NR] = res[c]["out"].T
    return out
